# revision 9
# baseline (speedup 1.0000x reference)
"""Trainium2 Bass kernel for causal multi-head attention.

Reference computation (B=2, T=2048, D=1024, H=16 heads, head_dim=64):
    q, k, v = x @ Wq, x @ Wk, x @ Wv         (per-head split)
    out = softmax(causal(q k^T / 8)) v  @ Wo

Sharding: 8 cores = 2 batches x 4 head-groups (4 heads each).  Each core
computes, for its batch b and its 4 heads:
    qT, kT [256, 2048] and v [2048, 256]  from the host-pre-transposed xT,
    transposed scores sT[tk, tq] = kT.T @ qT  (so softmax sums land on the
    matmul contraction axis and no on-chip transposes are ever needed),
    expS = exp(sT/8) * causal_mask,
    ctxT' [65, tq] = v'.T @ expS   with v' = [v | ones] so row 64 is the
    softmax denominator,
    ctxT_norm = ctxT * (1/rowsum)  (rank-1 PE broadcast of the reciprocal),
    partial_out [2048, 1024] = ctxT.T @ Wo[g*256:(g+1)*256, :].
Host sums the 4 partials per batch.

All matmuls run as float32r (TF32-like, full PE rate at N>=256).  Tiles that
feed the PE are allocated as float32r (walrus requires producer dtype to
match); PSUM accumulation stays fp32.

Scheduling: the attention i-loop rotates over all 4 heads (sT x4 then ctx x4)
so the PE never waits on a single exp, and the next chunk's QKV projection
matmuls are interleaved into the attention stream as fill work.
"""

import sys

if "/opt/trn_rl_repo" not in sys.path:
    sys.path.insert(0, "/opt/trn_rl_repo")

import numpy as np

B, T, D, H = 2, 2048, 1024, 16
HD = 64                   # head dim
NCORES = 8
GROUPS = 4                # head groups (cores per batch)
HPC = H // GROUPS         # heads per core = 4
DHC = HPC * HD            # per-core head columns = 256
NKB = D // 128            # 8 contraction blocks for the projections
NTB = T // 128            # 16 t-blocks
NCH = T // 512            # 4 tq chunks of 512

_CACHE = {}


def _build():
    import concourse.bacc as bacc
    import concourse.tile as tile
    from concourse import mybir

    fp32 = mybir.dt.float32
    fp32r = mybir.dt.float32r
    Exp = mybir.ActivationFunctionType.Exp

    nc = bacc.Bacc("TRN2", target_bir_lowering=False, debug=False,
                   num_devices=NCORES)

    xt_d = nc.dram_tensor("xt", [D, T], fp32, kind="ExternalInput")
    wq_d = nc.dram_tensor("wq", [D, DHC], fp32, kind="ExternalInput")
    wk_d = nc.dram_tensor("wk", [D, DHC], fp32, kind="ExternalInput")
    wv_d = nc.dram_tensor("wv", [D, DHC], fp32, kind="ExternalInput")
    wo_d = nc.dram_tensor("wo", [DHC, D], fp32, kind="ExternalInput")
    cm_d = nc.dram_tensor("cmask", [128, 1024], fp32, kind="ExternalInput")
    out_d = nc.dram_tensor("out", [T, D], fp32, kind="ExternalOutput")

    with tile.TileContext(nc) as tc:
        with (
            tc.tile_pool(name="consts", bufs=1) as consts,
            tc.tile_pool(name="xtp", bufs=2) as xtp,
            tc.tile_pool(name="big", bufs=1) as big,
            tc.tile_pool(name="es_pool", bufs=8) as es_pool,
            tc.tile_pool(name="small", bufs=3) as small,
            tc.tile_pool(name="outp", bufs=2) as outp,
            tc.tile_pool(name="psum", bufs=1, space="PSUM") as psum,
        ):
            wq_sb = consts.tile([128, NKB, DHC], fp32r)
            nc.sync.dma_start(out=wq_sb, in_=wq_d[:].rearrange("(k p) n -> p k n", p=128).bitcast(fp32r))
            wk_sb = consts.tile([128, NKB, DHC], fp32r)
            nc.sync.dma_start(out=wk_sb, in_=wk_d[:].rearrange("(k p) n -> p k n", p=128).bitcast(fp32r))
            wv_sb = consts.tile([128, NKB, DHC], fp32r)
            nc.sync.dma_start(out=wv_sb, in_=wv_d[:].rearrange("(k p) n -> p k n", p=128).bitcast(fp32r))
            wo_sb = consts.tile([128, 2, D], fp32r)
            nc.sync.dma_start(out=wo_sb, in_=wo_d[:].rearrange("(k p) n -> p k n", p=128).bitcast(fp32r))
            cm_sb = consts.tile([128, 1024], fp32r)
            nc.sync.dma_start(out=cm_sb, in_=cm_d[:].bitcast(fp32r))

            qt_sb = big.tile([128, 2, T], fp32r)
            kt_sb = big.tile([128, 2, T], fp32r)
            ct_sb = big.tile([128, 2, T], fp32r)
            vs_sb = big.tile([128, NTB, HPC, HD + 1], fp32r)
            # ones column of v' (cmask cols 512.. are all 1.0, dtype fp32r)
            nc.vector.tensor_copy(
                vs_sb[:, :, :, 64],
                cm_sb[:, 512:512 + NTB * HPC].rearrange("p (a b) -> p a b", a=NTB),
            )

            xt_r = xt_d[:].rearrange("(k p) t -> p k t", p=128).bitcast(fp32r)
            xt_c = [None] * NCH

            def load_xt(nj):
                xt_c[nj] = xtp.tile([128, NKB, 512], fp32r, tag="xt",
                                    name=f"xt{nj}")
                nc.sync.dma_start(out=xt_c[nj],
                                  in_=xt_r[:, :, nj * 512:(nj + 1) * 512])

            def qkv_quanta(nj):
                """8 closures, each one psum accumulation group of chunk nj."""
                cs = slice(nj * 512, (nj + 1) * 512)
                quanta = []

                def make_qk(wsb, dst, mb):
                    def go():
                        pq = psum.tile([128, 512], fp32, tag="mm", bufs=4,
                                       name=f"pq{nj}{mb}")
                        for kb in range(NKB):
                            nc.tensor.matmul(
                                pq,
                                wsb[:, kb, mb * 128:(mb + 1) * 128],
                                xt_c[nj][:, kb, :],
                                start=(kb == 0), stop=(kb == NKB - 1),
                            )
                        nc.vector.tensor_copy(dst[:, mb, cs], pq)
                    return go

                def make_v(tb):
                    def go():
                        pv = psum.tile([128, 512], fp32, tag="mm", bufs=4,
                                       name=f"pv{tb}")
                        for kb in range(NKB):
                            nc.tensor.matmul(
                                pv[:, 0:DHC],
                                xt_c[nj][:, kb, (tb - 4 * nj) * 128:(tb - 4 * nj + 1) * 128],
                                wv_sb[:, kb, :],
                                start=(kb == 0), stop=(kb == NKB - 1),
                            )
                        nc.vector.tensor_copy(
                            vs_sb[:, tb, :, 0:HD],
                            pv[:, 0:DHC].rearrange("p (h d) -> p h d", h=HPC),
                        )
                    return go

                for mb in range(2):
                    quanta.append(make_qk(wq_sb, qt_sb, mb))
                for mb in range(2):
                    quanta.append(make_qk(wk_sb, kt_sb, mb))
                for tb in range(4 * nj, 4 * nj + 4):
                    quanta.append(make_v(tb))
                return quanta

            # prologue: first chunk's x and QKV, dense
            load_xt(0)
            load_xt(1)
            for q in qkv_quanta(0):
                q()

            for nj in range(NCH):
                cs = slice(nj * 512, (nj + 1) * 512)
                nb = 4 * nj + 4     # causal: tk-blocks 0 .. nb-1
                if nj + 2 < NCH:
                    load_xt(nj + 2)
                fill = qkv_quanta(nj + 1) if nj + 1 < NCH else []
                fi = 0

                pcs = []
                for h in range(HPC):
                    pc = psum.tile([65, 512], fp32, tag="acc", bufs=4,
                                   name=f"pc{nj}{h}")
                    pcs.append(pc)

                for i in range(nb):
                    m = i - 4 * nj
                    # causal window: diagonal blocks only need cols >= wm
                    # (m==3 keeps N>=256 to stay at full fp32r rate)
                    wm = 0 if m < 0 else (128 * m if m < 3 else 256)
                    ess = []
                    for h in range(HPC):
                        mbh, ro = h >> 1, (h & 1) * 64
                        ps = psum.tile([128, 512], fp32, tag="mm", bufs=4,
                                       name=f"ps{nj}{h}{i}")
                        nc.tensor.matmul(
                            ps[:, wm:512],
                            kt_sb[ro:ro + 64, mbh, i * 128:(i + 1) * 128],
                            qt_sb[ro:ro + 64, mbh, nj * 512 + wm:(nj + 1) * 512],
                            start=True, stop=True,
                        )
                        es = es_pool.tile([128, 512], fp32r, tag="es",
                                          name=f"es{nj}{h}{i}")
                        nc.scalar.activation(out=es[:, wm:512], in_=ps[:, wm:512],
                                             func=Exp, scale=0.125)
                        if m >= 0:
                            # only the diagonal 128 cols (plus, for m==3, the
                            # 128 below-window cols) actually need masking
                            a = wm if m == 3 else 128 * m
                            nc.vector.tensor_mul(
                                es[:, a:128 * m + 128], es[:, a:128 * m + 128],
                                cm_sb[:, (3 - m) * 128 + a:512],
                            )
                        ess.append(es)
                    for h in range(HPC):
                        nc.tensor.matmul(
                            pcs[h][:, wm:512],
                            vs_sb[:, i, h, :],
                            ess[h][:, wm:512],
                            start=(i == 0), stop=(i == nb - 1),
                        )
                    # fill the ACT-bound pipeline with next chunk's QKV work
                    if fi < len(fill) and (i % 2 == 1 or nb - i <= len(fill) - fi):
                        fill[fi]()
                        fi += 1
                while fi < len(fill):
                    fill[fi]()
                    fi += 1

                for h in range(HPC):
                    mbh, ro = h >> 1, (h & 1) * 64
                    pc = pcs[h]
                    rs32 = small.tile([1, 512], fp32, tag="rs32",
                                      name=f"rs{nj}{h}")
                    nc.vector.reciprocal(out=rs32, in_=pc[64:65, :])
                    rc = small.tile([1, 512], fp32r, tag="rc", name=f"rc{nj}{h}")
                    with nc.allow_low_precision(reason="fp32r recip feeds fp32r PE bcast"):
                        nc.vector.tensor_copy(rc, rs32)
                    pb = psum.tile([64, 512], fp32, tag="mm", bufs=4,
                                   name=f"pb{nj}{h}")
                    nc.tensor.matmul(pb, cm_sb[0:1, 512:576], rc,
                                     start=True, stop=True)
                    bc_sb = small.tile([64, 512], fp32, tag="bc_sb",
                                       name=f"bc{nj}{h}")
                    nc.vector.tensor_copy(bc_sb, pb)
                    nc.vector.tensor_mul(ct_sb[ro:ro + 64, mbh, cs],
                                         pc[0:64, :], bc_sb)

                # ---- output projection for this chunk's t-blocks ----
                for tb in range(4 * nj, 4 * nj + 4):
                    ot = outp.tile([128, D], fp32, tag="ot", name=f"ot{tb}")
                    for nk in range(2):
                        po = psum.tile([128, 512], fp32, tag="mm", bufs=4,
                                       name=f"po{tb}{nk}")
                        for mb in range(2):
                            nc.tensor.matmul(
                                po,
                                ct_sb[:, mb, tb * 128:(tb + 1) * 128],
                                wo_sb[:, mb, nk * 512:(nk + 1) * 512],
                                start=(mb == 0), stop=(mb == 1),
                            )
                        nc.vector.tensor_copy(ot[:, nk * 512:(nk + 1) * 512], po)
                    nc.sync.dma_start(out=out_d[tb * 128:(tb + 1) * 128, :], in_=ot)

    nc.compile()
    return nc


def _causal_mask_block():
    # [128, 1024]: cols 0..383 = 0, cols 384..511 = upper-tri (p <= c-384),
    # cols 512.. = 1.  Slice [(3-m)*128 : (3-m)*128+512] masks a diagonal
    # tk-block at position m within a 512-wide tq chunk.
    m = np.zeros((128, 1024), np.float32)
    m[:, 512:] = 1.0
    m[:, 384:512] = np.triu(np.ones((128, 128), np.float32))
    return m


def _prepare_in_maps(x_q, Wq, Wk, Wv, Wo):
    x_q = np.asarray(x_q, np.float32)
    Wq = np.asarray(Wq, np.float32)
    Wk = np.asarray(Wk, np.float32)
    Wv = np.asarray(Wv, np.float32)
    Wo = np.asarray(Wo, np.float32)

    cmask = _causal_mask_block()
    xts = [np.ascontiguousarray(x_q[b].T) for b in range(B)]
    in_maps = []
    for c in range(NCORES):
        b, g = divmod(c, GROUPS)
        sl = slice(g * DHC, (g + 1) * DHC)
        in_maps.append({
            "xt": xts[b],
            "wq": np.ascontiguousarray(Wq[:, sl]),
            "wk": np.ascontiguousarray(Wk[:, sl]),
            "wv": np.ascontiguousarray(Wv[:, sl]),
            "wo": np.ascontiguousarray(Wo[sl, :]),
            "cmask": cmask,
        })
    return in_maps


def _gather(results):
    out = np.zeros((B, T, D), np.float32)
    for c in range(NCORES):
        out[c // GROUPS] += results[c]["out"]
    return out


def get_nc():
    if "nc" not in _CACHE:
        _CACHE["nc"] = _build()
    return _CACHE["nc"]


def kernel(x_q, Wq, Wk, Wv, Wo):
    from concourse.bass_utils import run_bass_kernel_spmd

    nc = get_nc()
    in_maps = _prepare_in_maps(x_q, Wq, Wk, Wv, Wo)
    res = run_bass_kernel_spmd(nc, in_maps, list(range(NCORES)))
    return _gather(res.results)


# revision 12
# speedup vs baseline: 1.2092x; 1.2092x over previous
"""Trainium2 Bass kernel for causal multi-head attention.

Reference computation (B=2, T=2048, D=1024, H=16 heads, head_dim=64):
    q, k, v = x @ Wq, x @ Wk, x @ Wv         (per-head split)
    out = softmax(causal(q k^T / 8)) v  @ Wo

Sharding: 8 cores = 2 batches x 4 head-groups (4 heads each).  Each core
computes, for its batch b and its 4 heads:
    qT, kT [256, 2048] and v [2048, 256]  from the host-pre-transposed xT,
    transposed scores sT[tk, tq] = kT.T @ qT  (so softmax sums land on the
    matmul contraction axis and no on-chip transposes are ever needed),
    expS = exp(sT/8) * causal_mask,
    ctxT' [65, tq] = v'.T @ expS   with v' = [v | ones] so row 64 is the
    softmax denominator,
    ctxT_norm = ctxT * (1/rowsum)  (rank-1 PE broadcast of the reciprocal),
    partial_out [2048, 1024] = ctxT.T @ Wo[g*256:(g+1)*256, :].
Host sums the 4 partials per batch.

All matmuls run as float32r (TF32-like, full PE rate at N>=256).  Tiles that
feed the PE are allocated as float32r (walrus requires producer dtype to
match); PSUM accumulation stays fp32.

Scheduling: the attention i-loop rotates over all 4 heads (sT x4 then ctx x4)
so the PE never waits on a single exp, and the next chunk's QKV projection
matmuls are interleaved into the attention stream as fill work.
"""

import sys

if "/opt/trn_rl_repo" not in sys.path:
    sys.path.insert(0, "/opt/trn_rl_repo")

import numpy as np

B, T, D, H = 2, 2048, 1024, 16
HD = 64                   # head dim
NCORES = 8
GROUPS = 4                # head groups (cores per batch)
HPC = H // GROUPS         # heads per core = 4
DHC = HPC * HD            # per-core head columns = 256
NKB = D // 128            # 8 contraction blocks for the projections
NTB = T // 128            # 16 t-blocks
NCH = T // 512            # 4 tq chunks of 512

_CACHE = {}


def _build():
    import concourse.bacc as bacc
    import concourse.tile as tile
    from concourse import mybir

    fp32 = mybir.dt.float32
    fp32r = mybir.dt.float32r
    Exp = mybir.ActivationFunctionType.Exp

    nc = bacc.Bacc("TRN2", target_bir_lowering=False, debug=False,
                   num_devices=NCORES)

    xt_d = nc.dram_tensor("xt", [D, T], fp32, kind="ExternalInput")
    wq_d = nc.dram_tensor("wq", [D, DHC], fp32, kind="ExternalInput")
    wk_d = nc.dram_tensor("wk", [D, DHC], fp32, kind="ExternalInput")
    wv_d = nc.dram_tensor("wv", [D, DHC], fp32, kind="ExternalInput")
    wo_d = nc.dram_tensor("wo", [DHC, D], fp32, kind="ExternalInput")
    cm_d = nc.dram_tensor("cmask", [128, 1024], fp32, kind="ExternalInput")
    out_d = nc.dram_tensor("out", [T, D], fp32, kind="ExternalOutput")

    with tile.TileContext(nc) as tc:
        with (
            tc.tile_pool(name="consts", bufs=1) as consts,
            tc.tile_pool(name="xtp", bufs=2) as xtp,
            tc.tile_pool(name="big", bufs=1) as big,
            tc.tile_pool(name="es_pool", bufs=8) as es_pool,
            tc.tile_pool(name="small", bufs=3) as small,
            tc.tile_pool(name="outp", bufs=2) as outp,
            tc.tile_pool(name="psum", bufs=1, space="PSUM") as psum,
        ):
            wq_sb = consts.tile([128, NKB, DHC], fp32r)
            nc.sync.dma_start(out=wq_sb, in_=wq_d[:].rearrange("(k p) n -> p k n", p=128).bitcast(fp32r))
            wk_sb = consts.tile([128, NKB, DHC], fp32r)
            nc.sync.dma_start(out=wk_sb, in_=wk_d[:].rearrange("(k p) n -> p k n", p=128).bitcast(fp32r))
            wv_sb = consts.tile([128, NKB, DHC], fp32r)
            nc.sync.dma_start(out=wv_sb, in_=wv_d[:].rearrange("(k p) n -> p k n", p=128).bitcast(fp32r))
            wo_sb = consts.tile([128, 2, D], fp32r)
            nc.sync.dma_start(out=wo_sb, in_=wo_d[:].rearrange("(k p) n -> p k n", p=128).bitcast(fp32r))
            cm_sb = consts.tile([128, 1024], fp32r)
            nc.sync.dma_start(out=cm_sb, in_=cm_d[:].bitcast(fp32r))

            qt_sb = big.tile([128, 2, T], fp32r)
            kt_sb = big.tile([128, 2, T], fp32r)
            ct_sb = big.tile([128, 2, T], fp32r)
            vs_sb = big.tile([128, NTB, HPC, HD + 1], fp32r)
            # ones column of v' (cmask cols 512.. are all 1.0, dtype fp32r)
            nc.vector.tensor_copy(
                vs_sb[:, :, :, 64],
                cm_sb[:, 512:512 + NTB * HPC].rearrange("p (a b) -> p a b", a=NTB),
            )

            xt_r = xt_d[:].rearrange("(k p) t -> p k t", p=128).bitcast(fp32r)
            xt_c = [None] * NCH
            # pcS[nj][h]: ctxT' drained to SBUF at end of chunk nj's attention
            pcS = [[None] * HPC for _ in range(NCH)]

            def load_xt(nj):
                xt_c[nj] = xtp.tile([128, NKB, 512], fp32r, tag="xt",
                                    name=f"xt{nj}")
                nc.sync.dma_start(out=xt_c[nj],
                                  in_=xt_r[:, :, nj * 512:(nj + 1) * 512])

            def qkv_halves(nj):
                """16 closures, each half a psum accumulation group (4 MMs)."""
                cs = slice(nj * 512, (nj + 1) * 512)
                quanta = []

                def make_qk(wsb, dst, mb):
                    pq = [None]

                    def go_a():
                        pq[0] = psum.tile([128, 512], fp32, tag="mm", bufs=4,
                                          name=f"pq{nj}{mb}")
                        for kb in range(4):
                            nc.tensor.matmul(
                                pq[0],
                                wsb[:, kb, mb * 128:(mb + 1) * 128],
                                xt_c[nj][:, kb, :],
                                start=(kb == 0), stop=False,
                            )

                    def go_b():
                        for kb in range(4, NKB):
                            nc.tensor.matmul(
                                pq[0],
                                wsb[:, kb, mb * 128:(mb + 1) * 128],
                                xt_c[nj][:, kb, :],
                                start=False, stop=(kb == NKB - 1),
                            )
                        nc.vector.tensor_copy(dst[:, mb, cs], pq[0])
                    return go_a, go_b

                def make_v(tb):
                    pv = [None]

                    def go_a():
                        pv[0] = psum.tile([128, 512], fp32, tag="mm", bufs=4,
                                          name=f"pv{tb}")
                        for kb in range(4):
                            nc.tensor.matmul(
                                pv[0][:, 0:DHC],
                                xt_c[nj][:, kb, (tb - 4 * nj) * 128:(tb - 4 * nj + 1) * 128],
                                wv_sb[:, kb, :],
                                start=(kb == 0), stop=False,
                            )

                    def go_b():
                        for kb in range(4, NKB):
                            nc.tensor.matmul(
                                pv[0][:, 0:DHC],
                                xt_c[nj][:, kb, (tb - 4 * nj) * 128:(tb - 4 * nj + 1) * 128],
                                wv_sb[:, kb, :],
                                start=False, stop=(kb == NKB - 1),
                            )
                        nc.vector.tensor_copy(
                            vs_sb[:, tb, :, 0:HD],
                            pv[0][:, 0:DHC].rearrange("p (h d) -> p h d", h=HPC),
                        )
                    return go_a, go_b

                for mb in range(2):
                    quanta.extend(make_qk(wq_sb, qt_sb, mb))
                for mb in range(2):
                    quanta.extend(make_qk(wk_sb, kt_sb, mb))
                for tb in range(4 * nj, 4 * nj + 4):
                    quanta.extend(make_v(tb))
                return quanta

            def norm_fill(nj, h):
                """normalize head h of chunk nj from the SBUF-drained ctxT'."""
                def go():
                    mbh, ro = h >> 1, (h & 1) * 64
                    src = pcS[nj][h]
                    rs32 = small.tile([1, 512], fp32, tag="rs32",
                                      name=f"rs{nj}{h}")
                    nc.vector.reciprocal(out=rs32, in_=src[64:65, :])
                    rc = small.tile([1, 512], fp32r, tag="rc",
                                    name=f"rc{nj}{h}")
                    with nc.allow_low_precision(reason="fp32r recip for PE bcast"):
                        nc.vector.tensor_copy(rc, rs32)
                    pb = psum.tile([64, 512], fp32, tag="mm", bufs=4,
                                   name=f"pb{nj}{h}")
                    nc.tensor.matmul(pb, cm_sb[0:1, 512:576], rc,
                                     start=True, stop=True)
                    nc.vector.tensor_mul(
                        ct_sb[ro:ro + 64, mbh, nj * 512:(nj + 1) * 512],
                        src[0:64, :], pb)
                return go

            def outproj_fill(nj, tb):
                def go():
                    ot = outp.tile([128, D], fp32, tag="ot", name=f"ot{tb}")
                    for nk in range(2):
                        po = psum.tile([128, 512], fp32, tag="mm", bufs=4,
                                       name=f"po{tb}{nk}")
                        for mb in range(2):
                            nc.tensor.matmul(
                                po,
                                ct_sb[:, mb, tb * 128:(tb + 1) * 128],
                                wo_sb[:, mb, nk * 512:(nk + 1) * 512],
                                start=(mb == 0), stop=(mb == 1),
                            )
                        nc.vector.tensor_copy(ot[:, nk * 512:(nk + 1) * 512], po)
                    nc.sync.dma_start(out=out_d[tb * 128:(tb + 1) * 128, :],
                                      in_=ot)
                return go

            def tail_fills(nj):
                fills = [norm_fill(nj, h) for h in range(HPC)]
                fills += [outproj_fill(nj, tb)
                          for tb in range(4 * nj, 4 * nj + 4)]
                return fills

            # prologue: first chunk's x and QKV, dense
            load_xt(0)
            load_xt(1)
            for q in qkv_halves(0):
                q()

            for nj in range(NCH):
                nb = 4 * nj + 4     # causal: tk-blocks 0 .. nb-1
                if nj + 2 < NCH:
                    load_xt(nj + 2)
                fill = list(qkv_halves(nj + 1)) if nj + 1 < NCH else []
                if nj >= 1:
                    # previous chunk's normalization + output projection,
                    # interleaved ahead of the QKV fill
                    import itertools
                    prev = tail_fills(nj - 1)
                    mixed = []
                    for x, y in itertools.zip_longest(prev[4:], fill[:4]):
                        if x is not None:
                            mixed.append(x)
                        if y is not None:
                            mixed.append(y)
                    fill = prev[:4] + mixed + fill[4:]
                fi = 0

                pcs = []
                for h in range(HPC):
                    pc = psum.tile([65, 512], fp32, tag="acc", bufs=4,
                                   name=f"pc{nj}{h}")
                    pcs.append(pc)

                per_iter = max(1, -(-len(fill) // nb))
                for i in range(nb):
                    m = i - 4 * nj
                    # causal window: diagonal blocks only need cols >= wm
                    # (m==3 keeps N>=256 to stay at full fp32r rate)
                    wm = 0 if m < 0 else (128 * m if m < 3 else 256)
                    ess = []
                    for h in range(HPC):
                        mbh, ro = h >> 1, (h & 1) * 64
                        ps = psum.tile([128, 512], fp32, tag="mm", bufs=4,
                                       name=f"ps{nj}{h}{i}")
                        nc.tensor.matmul(
                            ps[:, wm:512],
                            kt_sb[ro:ro + 64, mbh, i * 128:(i + 1) * 128],
                            qt_sb[ro:ro + 64, mbh, nj * 512 + wm:(nj + 1) * 512],
                            start=True, stop=True,
                        )
                        es = es_pool.tile([128, 512], fp32r, tag="es",
                                          name=f"es{nj}{h}{i}")
                        nc.scalar.activation(out=es[:, wm:512], in_=ps[:, wm:512],
                                             func=Exp, scale=0.125)
                        if m >= 0:
                            # only the diagonal 128 cols (plus, for m==3, the
                            # 128 below-window cols) actually need masking
                            a = wm if m == 3 else 128 * m
                            nc.vector.tensor_mul(
                                es[:, a:128 * m + 128], es[:, a:128 * m + 128],
                                cm_sb[:, (3 - m) * 128 + a:512],
                            )
                        ess.append(es)
                    for h in range(HPC):
                        nc.tensor.matmul(
                            pcs[h][:, wm:512],
                            vs_sb[:, i, h, :],
                            ess[h][:, wm:512],
                            start=(i == 0), stop=(i == nb - 1),
                        )
                    # fill the ACT-bound pipeline with deferred + QKV work
                    for _ in range(per_iter):
                        if fi < len(fill):
                            fill[fi]()
                            fi += 1
                while fi < len(fill):
                    fill[fi]()
                    fi += 1

                # drain ctxT' to SBUF so the PSUM acc banks free up and the
                # normalization can run as fill work in the next chunk
                for h in range(HPC):
                    dst = small.tile([65, 512], fp32, tag="pcs", bufs=8,
                                     name=f"pcS{nj}{h}")
                    nc.vector.tensor_copy(dst, pcs[h])
                    pcS[nj][h] = dst

            # last chunk's tail has no next attention to hide in
            for go in tail_fills(NCH - 1):
                go()

    nc.compile()
    return nc


def _causal_mask_block():
    # [128, 1024]: cols 0..383 = 0, cols 384..511 = upper-tri (p <= c-384),
    # cols 512.. = 1.  Slice [(3-m)*128 : (3-m)*128+512] masks a diagonal
    # tk-block at position m within a 512-wide tq chunk.
    m = np.zeros((128, 1024), np.float32)
    m[:, 512:] = 1.0
    m[:, 384:512] = np.triu(np.ones((128, 128), np.float32))
    return m


def _prepare_in_maps(x_q, Wq, Wk, Wv, Wo):
    x_q = np.asarray(x_q, np.float32)
    Wq = np.asarray(Wq, np.float32)
    Wk = np.asarray(Wk, np.float32)
    Wv = np.asarray(Wv, np.float32)
    Wo = np.asarray(Wo, np.float32)

    cmask = _causal_mask_block()
    xts = [np.ascontiguousarray(x_q[b].T) for b in range(B)]
    in_maps = []
    for c in range(NCORES):
        b, g = divmod(c, GROUPS)
        sl = slice(g * DHC, (g + 1) * DHC)
        in_maps.append({
            "xt": xts[b],
            "wq": np.ascontiguousarray(Wq[:, sl]),
            "wk": np.ascontiguousarray(Wk[:, sl]),
            "wv": np.ascontiguousarray(Wv[:, sl]),
            "wo": np.ascontiguousarray(Wo[sl, :]),
            "cmask": cmask,
        })
    return in_maps


def _gather(results):
    out = np.zeros((B, T, D), np.float32)
    for c in range(NCORES):
        out[c // GROUPS] += results[c]["out"]
    return out


def get_nc():
    if "nc" not in _CACHE:
        _CACHE["nc"] = _build()
    return _CACHE["nc"]


def kernel(x_q, Wq, Wk, Wv, Wo):
    from concourse.bass_utils import run_bass_kernel_spmd

    nc = get_nc()
    in_maps = _prepare_in_maps(x_q, Wq, Wk, Wv, Wo)
    res = run_bass_kernel_spmd(nc, in_maps, list(range(NCORES)))
    return _gather(res.results)
